# revision 18
# baseline (speedup 1.0000x reference)
"""Trainium2 Bass kernel: batched 4-point DLT homography (closed-form solve).

Contract: kernel(pts_1_tile, pred_h4p_tile) -> [B, 3, 3] float32, with
B = 524288 split across 8 NeuronCores (batch-parallel, no communication).

Math (per batch element, points p=0..3 with src (x_p,y_p), dst (X_p,Y_p)):
the DLT system rows are
    x h0 + y h1 + h2 = X (1 + x h6 + y h7)
    x h3 + y h4 + h5 = Y (1 + x h6 + y h7)
Eliminating (h0,h1,h2) from the four X-equations via the left null vector n
of M = [(x_p, y_p, 1)] gives one linear equation in (h6,h7); same for the
Y-equations. Solve the 2x2, back out the rest in closed form.

Layout: per-core 65536 elements as [128 partitions, 512 free], two
asymmetric chunks so chunk-0 compute starts early.  The HOST pre-transposes
inputs to planar component planes packed per-chunk (contiguous 2D DMA at
full rate) and post-transposes the planar fp16 output back to [B,3,3]
fp32 (+ the constant ones column) — so the device does zero shuffling:
no deinterleave, no output staging copies.  ScalarE/GPSIMD stay idle
(their SBUF traffic would slow concurrent DVE ops); VectorE runs all math
as fp16-2x multi-plane ops over [128,fc] planes in a no-reuse slab.
Dot products use sum(n * {1,x,y}) = 0:  a/b/c = sum_{p>=1} n_p {1,x,y}_p
(U_p - U_0), 9 products per axis, no n0.  One reciprocal_approx_fast
(~18 bits, plenty next to fp16) covers 1/n3 and 1/det in one op.
"""
import sys

for _p in ("/opt/trn_rl_repo", "/root/.axon_site/_ro/trn_rl_repo"):
    if _p not in sys.path:
        sys.path.append(_p)

import numpy as np

import concourse.bass as bass
import concourse.mybir as mybir
from concourse import bacc
from concourse.tile import TileContext
from concourse.bass_utils import run_bass_kernel_spmd

N_CORES = 8
B_TOTAL = 524288
PER_CORE = B_TOTAL // N_CORES  # 65536
PARTS = 128
F = PER_CORE // PARTS  # 512
CHUNKS = [96, 416]
FP32 = mybir.dt.float32
FP16 = mybir.dt.float16

ADD = mybir.AluOpType.add
SUB = mybir.AluOpType.subtract
MUL = mybir.AluOpType.mult

# component order of the planar input planes: x0..x3, y0..y3
CORDER = [0, 2, 4, 6, 1, 3, 5, 7]


class _Slab:
    """Bump allocator, F-plane units, no reuse (avoids WAR semaphores)."""

    def __init__(self, nplanes):
        self.off = 0
        self.nplanes = nplanes

    def alloc(self, n):
        off = self.off
        self.off += n
        if self.off > self.nplanes:
            raise RuntimeError(f"slab OOM at {self.off}/{self.nplanes}")
        return off


OPLOG = {}


def _build():
    OPLOG.clear()
    nc = bacc.Bacc(None, target_bir_lowering=False, debug=True)
    # planar, chunk-blocked: per partition [8 pts planes | 8 pred planes]
    # per chunk, all packed in one tensor => one input DMA per chunk
    inp = nc.dram_tensor("inp", [PARTS, 16 * F], FP16, kind="ExternalInput")
    out = nc.dram_tensor("out", [PARTS, 8 * F], FP16, kind="ExternalOutput")

    N32 = 4   # fp32 slab: [n3_32, det_32, rD_32, rdet_32]
    NP = 137  # fp16 compute-plane slab (no reuse)

    with TileContext(nc) as tc:
        with tc.tile_pool(name="st", bufs=1) as pool:
            slabs = {}
            for c, fc in enumerate(CHUNKS):
                slabs[c] = (
                    pool.tile([PARTS, NP * fc], FP16, tag=f"sp{c}", name=f"sp{c}"),
                    pool.tile([PARTS, N32 * fc], FP32, tag=f"s32_{c}", name=f"s32_{c}"),
                )

            # xv/pp are the first 16 slab planes; DMA inputs straight in.
            # All input DMAs up front, chunk 0 first (smallest).
            off = 0
            for c, fc in enumerate(CHUNKS):
                # pts block first: the first V ops only need xv
                nc.sync.dma_start(
                    out=slabs[c][0][:, : 8 * fc],
                    in_=inp[:, off : off + 8 * fc],
                )
                nc.sync.dma_start(
                    out=slabs[c][0][:, 8 * fc : 16 * fc],
                    in_=inp[:, off + 8 * fc : off + 16 * fc],
                )
                off += 16 * fc

            off = 0
            for c, fc in enumerate(CHUNKS):
                slabp, slab32 = slabs[c]
                sa = _Slab(NP)

                def R32(o, n):
                    return slab32[:, o * fc : (o + n) * fc]

                def R(o, n):
                    return slabp[:, o * fc : (o + n) * fc]

                def V(o, n):
                    return R(o, n).rearrange("p (c f) -> p c f", f=fc)

                def PL(o):
                    return R(o, 1)

                def BC(o, k):
                    return PL(o).unsqueeze(1).broadcast_to((PARTS, k, fc))

                def vtt(o, a, b, op, desc=""):
                    ins = nc.vector.tensor_tensor(out=o, in0=a, in1=b, op=op)
                    OPLOG[ins.ins.name] = desc or "tt"

                def scp(o, i, desc="scp"):
                    ins = nc.scalar.copy(out=o, in_=i)
                    OPLOG[ins.ins.name] = desc

                def stt(o, in0, scalar, in1, op0, op1, desc="stt"):
                    ins = nc.vector.scalar_tensor_tensor(
                        out=o, in0=in0, scalar=scalar, in1=in1, op0=op0, op1=op1
                    )
                    OPLOG[ins.ins.name] = desc

                xv = sa.alloc(8)  # [x0,x1,x2,x3,y0,y1,y2,y3]  (DMA'd)
                pp = sa.alloc(8)  # pred planar, same order     (DMA'd)
                uu = sa.alloc(8)  # [X0..X3, Y0..Y3] = xv + pp

                # diffs in rotation-extended layout:
                # dd = [dx1,dx2,dx3,dx1',dx2' | dy1,dy2,dy3,dy1',dy2']
                dd = sa.alloc(10)
                xv2 = R(xv, 8).rearrange("p (a q f) -> p a q f", a=2, q=4)
                dv = R(dd, 10).rearrange("p (a q f) -> p a q f", a=2, q=5)
                vtt(dv[:, :, 0:3, :],
                    xv2[:, :, 1:4, :],
                    xv2[:, :, 0, :].unsqueeze(2).broadcast_to((PARTS, 2, 3, fc)),
                    SUB, "diffs")
                DX1, DX2, DY1, DY2 = dd, dd + 1, dd + 5, dd + 6
                # uu = xv + pred; also gives ScalarE time for the dup copy
                vtt(R(uu, 8), R(xv, 8), R(pp, 8), ADD, "uadd")
                scp(dv[:, :, 3:5, :], dv[:, :, 0:2, :], desc="dupdd")

                # n = cross(dx, dy) as two rotated 3-plane muls:
                # pa = (dx2 dy3, dx3 dy1, dx1 dy2), pb = (dx3 dy2, dx1 dy3, dx2 dy1)
                pa = sa.alloc(3)
                pb = sa.alloc(3)
                vtt(R(pa, 3), V(dd, 10)[:, 1:4, :], V(dd, 10)[:, 7:10, :],
                    MUL, "pa")
                vtt(R(pb, 3), V(dd, 10)[:, 2:5, :], V(dd, 10)[:, 6:9, :],
                    MUL, "pb")
                ns = sa.alloc(3)  # fp16 [n1,n2,n3]
                vtt(R(ns, 3), R(pa, 3), R(pb, 3), SUB, "nsub")
                # UD_p = U_p - U_0 (p=1..3)
                uu2 = R(uu, 8).rearrange("p (a q f) -> p a q f", a=2, q=4)
                ud = sa.alloc(6)  # [UX1,UX2,UX3,UY1,UY2,UY3]
                vtt(R(ud, 6).rearrange("p (a q f) -> p a q f", a=2, q=3),
                    uu2[:, :, 1:4, :],
                    uu2[:, :, 0, :].unsqueeze(2).broadcast_to((PARTS, 2, 3, fc)),
                    SUB, "udiff")
                # fp32 n3 for the reciprocal (sub done at fp32 from fp16 in)
                vtt(R32(0, 1), PL(pa + 2), PL(pb + 2), SUB, "n3_32")

                # dots via sum(n)=0 (p=1..3), z_p = n_p UD_p, q_p = z_p x_p,
                # r_p = z_p y_p.  Group order chosen so the 3-term sums land
                # as ssY=[cY,aY,bY] and ssX=[bX,cX,aX] for rotated 2x2 muls.
                # Y axis first: its dup copy hides behind the X-axis ops.
                zy = sa.alloc(9)   # [r1..3 | z1..3 | q1..3]
                vtt(V(zy, 9)[:, 3:6, :], V(ns, 3), V(ud, 6)[:, 3:6, :],
                    MUL, "zY")
                vtt(V(zy, 9)[:, 6:9, :], V(zy, 9)[:, 3:6, :],
                    V(xv, 8)[:, 1:4, :], MUL, "qY")
                vtt(V(zy, 9)[:, 0:3, :], V(zy, 9)[:, 3:6, :],
                    V(xv, 8)[:, 5:8, :], MUL, "rY")
                ss = sa.alloc(9)  # [bX,cX,aX,bX' | cY,aY,bY,cY',aY']
                s2y = sa.alloc(3)
                gy = R(zy, 9).rearrange("p (g q f) -> p g q f", g=3, q=3)
                vtt(R(s2y, 3).rearrange("p (g f) -> p g f", g=3),
                    gy[:, :, 0, :], gy[:, :, 1, :], ADD, "s2y")
                vtt(V(ss, 9)[:, 4:7, :],
                    R(s2y, 3).rearrange("p (g f) -> p g f", g=3),
                    gy[:, :, 2, :], ADD, "ssY")
                scp(V(ss, 9)[:, 7:9, :], V(ss, 9)[:, 4:6, :], desc="dupssY")
                # X axis: [q1..3 | r1..3 | z1..3]
                zx = sa.alloc(9)
                vtt(V(zx, 9)[:, 6:9, :], V(ns, 3), V(ud, 6)[:, 0:3, :],
                    MUL, "zX")
                vtt(V(zx, 9)[:, 0:3, :], V(zx, 9)[:, 6:9, :],
                    V(xv, 8)[:, 1:4, :], MUL, "qX")
                vtt(V(zx, 9)[:, 3:6, :], V(zx, 9)[:, 6:9, :],
                    V(xv, 8)[:, 5:8, :], MUL, "rX")
                s2x = sa.alloc(3)
                gx = R(zx, 9).rearrange("p (g q f) -> p g q f", g=3, q=3)
                vtt(R(s2x, 3).rearrange("p (g f) -> p g f", g=3),
                    gx[:, :, 0, :], gx[:, :, 1, :], ADD, "s2x")
                vtt(V(ss, 9)[:, 0:3, :],
                    R(s2x, 3).rearrange("p (g f) -> p g f", g=3),
                    gx[:, :, 2, :], ADD, "ssX")
                # bX' dup recomputed on V (1 plane, avoids an S handoff stall)
                vtt(PL(ss + 3), PL(s2x), gx[:, 0, 2, :], ADD, "ssXd")

                # 2x2: pc = (bX cY, cX aY, aX bY), pd = (cX bY, aX cY, bX aY)
                pc = sa.alloc(3)
                pd = sa.alloc(3)
                vtt(R(pc, 3), V(ss, 9)[:, 0:3, :], V(ss, 9)[:, 4:7, :],
                    MUL, "pc")
                vtt(R(pd, 3), V(ss, 9)[:, 1:4, :], V(ss, 9)[:, 6:9, :],
                    MUL, "pd")
                # det at fp32 (adjacent to n3_32), h6n/h7n at fp16
                vtt(R32(1, 1), PL(pc), PL(pd), SUB, "det32")
                dt67 = sa.alloc(2)
                vtt(V(dt67, 2), V(pc, 3)[:, 1:3, :], V(pd, 3)[:, 1:3, :], SUB,
                    "dt67")

                # fused reciprocal over [n3_32, det_32] -> [rD_32, rdet_32]
                ins = nc.vector.reciprocal_approx_fast(
                    out=R32(2, 2), in_=R32(0, 2)
                )
                OPLOG[ins.ins.name] = "recip"
                rc = sa.alloc(2)  # fp16 [rD, rdet]
                ins = nc.vector.tensor_scalar_add(
                    out=R(rc, 2), in0=R32(2, 2), scalar1=0.0
                )
                OPLOG[ins.ins.name] = "rcast"

                # output staging: planar fp16 planes [h0..h7]
                oo = sa.alloc(8)
                ov = V(oo, 8)
                vtt(ov[:, 6:8, :], V(dt67, 2), BC(rc + 1, 2), MUL, "h67")
                # h6/h7 fly out mid-compute (contiguous columns)
                nc.sync.dma_start(
                    out=out[:, off + 6 * fc : off + 8 * fc],
                    in_=R(oo + 6, 2),
                )

                # m = [x_p h6 (p=0..2), y_p h7 (p=0..2)]   (one fused op)
                m = sa.alloc(6)
                h67b = (
                    ov[:, 6:8, :].unsqueeze(2).broadcast_to((PARTS, 2, 3, fc))
                )
                vtt(R(m, 6).rearrange("p (a q f) -> p a q f", a=2, q=3),
                    xv2[:, :, 0:3, :], h67b, MUL, "m12")
                # w = (m1 + 1) + m2  (fused +1)
                w = sa.alloc(3)
                stt(R(w, 3), R(m, 3), 1.0, R(m + 3, 3), ADD, ADD, "wfuse")
                # XW_p = w_p X_p, YW_p = w_p Y_p (p=0..2)   (one fused op)
                xw = sa.alloc(6)  # [XW0,XW1,XW2,YW0,YW1,YW2]
                wb = (
                    R(w, 3).rearrange("p (q f) -> p q f", f=fc)
                    .unsqueeze(1).broadcast_to((PARTS, 2, 3, fc))
                )
                vtt(R(xw, 6).rearrange("p (a q f) -> p a q f", a=2, q=3),
                    wb, uu2[:, :, 0:3, :], MUL, "xwyw")

                # PQ = (XW1-XW0, XW2-XW0, YW1-YW0, YW2-YW0)
                pq = sa.alloc(4)
                xwv = R(xw, 6).rearrange("p (a b f) -> p a b f", a=2, b=3)
                vtt(
                    R(pq, 4).rearrange("p (a b f) -> p a b f", a=2, b=2),
                    xwv[:, :, 1:3, :],
                    xwv[:, :, 0, :].unsqueeze(2).broadcast_to((PARTS, 2, 2, fc)),
                    SUB,
                    "PQ",
                )

                # pe/pf with strided dsts so hn = [h0n,h1n,h3n,h4n]
                pe = sa.alloc(4)
                pf = sa.alloc(4)
                pqv = V(pq, 4)
                vtt(V(pe, 4)[:, 0:3:2, :], pqv[:, 0:3:2, :], BC(DY2, 2), MUL,
                    "pe01")
                vtt(V(pe, 4)[:, 1:4:2, :], pqv[:, 1:4:2, :], BC(DX1, 2), MUL,
                    "pe23")
                vtt(V(pf, 4)[:, 0:3:2, :], pqv[:, 1:4:2, :], BC(DY1, 2), MUL,
                    "pf01")
                vtt(V(pf, 4)[:, 1:4:2, :], pqv[:, 0:3:2, :], BC(DX2, 2), MUL,
                    "pf23")
                hn = sa.alloc(4)  # [h0n, h1n, h3n, h4n]
                vtt(R(hn, 4), R(pe, 4), R(pf, 4), SUB, "hn")
                # h = hn * rD into output planes (0,1) and (3,4) in one op
                rcb = (
                    PL(rc).unsqueeze(1).unsqueeze(1)
                    .broadcast_to((PARTS, 2, 2, fc))
                )
                vtt(R(oo, 6).rearrange("p (g q f) -> p g q f", g=2, q=3)[:, :, 0:2, :],
                    R(hn, 4).rearrange("p (g q f) -> p g q f", g=2, q=2),
                    rcb, MUL, "hg")

                # h2 = XW0 - x0 h0 - y0 h1 ; h5 = YW0 - x0 h3 - y0 h4
                ee = sa.alloc(4)  # (x0 h0, y0 h1, x0 h3, y0 h4)
                xy0 = V(xv, 8)[:, 0:5:4, :]  # (x0, y0)
                vtt(V(ee, 4)[:, 0:2, :], xy0, ov[:, 0:2, :], MUL, "ee1")
                vtt(V(ee, 4)[:, 2:4, :], xy0, ov[:, 3:5, :], MUL, "ee2")
                s1 = sa.alloc(2)
                eev = V(ee, 4)
                vtt(V(s1, 2), V(xw, 6)[:, 0:4:3, :], eev[:, 0:3:2, :], SUB,
                    "s1")
                vtt(ov[:, 2:6:3, :], V(s1, 2), eev[:, 1:4:2, :], SUB, "h25")
                nc.sync.dma_start(
                    out=out[:, off : off + 6 * fc], in_=R(oo, 6)
                )
                off += 8 * fc
    nc.finalize()
    return nc


_NC_CACHE = {}


def _get_nc():
    if "nc" not in _NC_CACHE:
        _NC_CACHE["nc"] = _build()
    return _NC_CACHE["nc"]


def _pack(pts, prd):
    """2x [PER_CORE, 8] fp16 -> [PARTS, 16*F] planar chunk-blocked."""
    a = pts.reshape(PARTS, F, 8)[:, :, CORDER].transpose(0, 2, 1)  # [p,c,f]
    b = prd.reshape(PARTS, F, 8)[:, :, CORDER].transpose(0, 2, 1)
    lo = 0
    blocks = []
    for fc in CHUNKS:
        blocks.append(a[:, :, lo : lo + fc].reshape(PARTS, 8 * fc))
        blocks.append(b[:, :, lo : lo + fc].reshape(PARTS, 8 * fc))
        lo += fc
    return np.ascontiguousarray(np.concatenate(blocks, axis=1))


def _unpack(o):
    """[PARTS, 8*F] planar chunk-blocked fp16 -> [PER_CORE, 8] fp32."""
    cols = np.empty((PARTS, 8, F), np.float16)
    lo = co = 0
    for fc in CHUNKS:
        cols[:, :, lo : lo + fc] = o[:, co : co + 8 * fc].reshape(PARTS, 8, fc)
        lo += fc
        co += 8 * fc
    return cols.transpose(0, 2, 1).reshape(PER_CORE, 8).astype(np.float32)


def kernel(pts_1_tile, pred_h4p_tile, _trace=False):
    pts = np.asarray(pts_1_tile).reshape(B_TOTAL, 8).astype(np.float16)
    prd = np.asarray(pred_h4p_tile).reshape(B_TOTAL, 8).astype(np.float16)
    nc = _get_nc()
    in_maps = [
        {
            "inp": _pack(pts[i * PER_CORE : (i + 1) * PER_CORE],
                         prd[i * PER_CORE : (i + 1) * PER_CORE]),
        }
        for i in range(N_CORES)
    ]
    res = run_bass_kernel_spmd(nc, in_maps, list(range(N_CORES)), trace=_trace)
    H = np.ones((B_TOTAL, 9), np.float32)
    for i in range(N_CORES):
        H[i * PER_CORE : (i + 1) * PER_CORE, :8] = _unpack(res.results[i]["out"])
    H = H.reshape(B_TOTAL, 3, 3)
    if _trace:
        return H, res
    return H
